# revision 37
# baseline (speedup 1.0000x reference)
"""Trainium2 Bass kernel for nn_AttnAdapter: GQA attention with RoPE,
region-based enhance/suppress score scaling, causal mask, o_proj.

Sharding: tensor-parallel over heads across 8 NeuronCores. Core d holds
q-heads 4d..4d+3 (wq rows), kv-head d (wk/wv rows), and wo columns
512d..512(d+1). Each core computes a full [S, D] partial of the output;
the host sums the 8 partials (the TP all-reduce, done at unshard time).

Structure (per core, ~420us HW vs 791us baseline):
- bf16 operands everywhere on-chip (fp32 PSUM accumulation); weights and
  x are SBUF-resident, loaded exactly once, with a laddered DMA issue
  order so the first matmuls start ~12us into the kernel.
- One fused loop over the four 512-row q blocks j: projections+RoPE ->
  attention -> o_proj, so exp/evacuations/DMA always overlap dense PE
  work from a neighboring phase.
- PSUM's 8 banks are partitioned by tag (4 singles S0-S3 + 2 doubles
  D0/D1) such that phase A(j+1) (on S0-S2) is always schedulable while
  B(j)'s exp backlog drains (on D0/D1) and C(j) evacuates: the PE
  "detours" into next-block projections instead of idling on softmax.
- Attention is head-paired: one [128,1024] score tile and one 2D-AP exp
  activation cover two heads per sk tile (the enhance/suppress scale is
  per key position, folded into exp's per-partition scale operand);
  causal-diagonal tiles are column-trimmed. The softmax denominator
  accumulates on the vector engine, off the PE. For j=3 (no next block
  to detour into) both head pairs run as interleaved pipelines.
- o_proj accumulates [128,1024] n-pairs, evacuated once per pair on the
  vector engine; the host sums the 8 cores' bf16 partials (the TP
  all-reduce at unshard time).
"""

import math

import numpy as np

# ---- problem constants (hardcoded; kernel.py must be self-contained) ----
S = 2048          # sequence length
D = 4096          # model dim
HD = 128          # head dim
NCORES = 8
QH = 4            # q heads per core
SYS_LEN, IMG_LEN = 35, 576
BOUND = SYS_LEN + IMG_LEN          # 611
ENH, SUP = 1.5, 0.5
ROPE_BASE = 10000.0

J = 4             # sq tiles of 512
NSK = 16          # sk tiles of 128
DCH = 32          # D chunks of 128
XE = 8            # x eighth-blocks per j (4 d-chunks each)

_CACHE = {}


def _host_constants():
    import ml_dtypes
    bf16 = ml_dtypes.bfloat16

    inv_freq = 1.0 / (ROPE_BASE ** (np.arange(0, HD, 2, dtype=np.float32) / HD))
    pos = np.arange(S, dtype=np.float32)
    freqs = pos[:, None] * inv_freq[None, :]              # [S, 64]
    emb = np.concatenate([freqs, freqs], axis=-1)         # [S, 128]
    cosT = np.ascontiguousarray(np.cos(emb).T).astype(bf16)  # [128, S]
    sinT = np.ascontiguousarray(np.sin(emb).T).astype(bf16)

    # rotate_half as a matmul: rot = R @ q (in [hd, s] layout).
    # matmul(out, lhsT, rhs) = lhsT.T @ rhs, so feed RT = R.T.
    RT = np.zeros((HD, HD), dtype=np.float32)
    half = HD // 2
    for c in range(half):
        RT[c + half, c] = -1.0      # rot[c] = -q[c+64]
    for c in range(half, HD):
        RT[c - half, c] = 1.0       # rot[c] = q[c-64]

    ident = np.eye(HD, dtype=np.float32).astype(bf16)

    # Diagonal-tile causal masks, T layout [sk 128, sq 512]:
    # tile (i=4j+delta, j): valid (keep) iff sq >= sk  <=>  f >= 128*delta + p
    masks = np.zeros((HD, 4 * 512), dtype=np.float32)
    p = np.arange(128)[:, None]
    f = np.arange(512)[None, :]
    for delta in range(4):
        masks[:, delta * 512:(delta + 1) * 512] = (f >= 128 * delta + p)
    masks = masks.astype(bf16)

    # key_scale in partition layout per sk-tile: ksT[p, i] = scale(128*i+p)
    kpos = np.arange(S)
    key_scale = np.where(kpos < SYS_LEN, SUP,
                         np.where(kpos < BOUND, ENH, 1.0)).astype(np.float32)
    ksT = np.ascontiguousarray(key_scale[:5 * 128].reshape(5, 128).T)  # [128, 5]

    constsb = np.zeros((HD, 385), dtype=bf16)
    constsb[:, 0:128] = RT.astype(bf16)
    constsb[:, 128:256] = ident
    constsb[:, 256] = np.ones(HD, dtype=np.float32).astype(bf16)
    constsb[0, 257:385] = np.ones(HD, dtype=np.float32).astype(bf16)
    constsf = np.ones((HD, 6), dtype=np.float32)
    constsf[:, 0:5] = ksT
    return dict(cosT=cosT, sinT=sinT, masks=masks,
                constsb=constsb, constsf=constsf,
                ones_colr=np.ones((HD, 1), dtype=np.float32))


def _build_bass():
    import concourse.bass as bass
    import concourse.mybir as mybir
    from concourse.tile import TileContext
    from contextlib import ExitStack

    f32 = mybir.dt.float32
    f32r = mybir.dt.float32r
    bf16 = mybir.dt.bfloat16

    nc = bass.Bass()
    # x eighths: xh[j, e, p, 512*c4 + f] = x.T[128*(4e+c4)+p, 512j+f]
    xh_d = nc.dram_tensor("xh", [J, XE, 128, 4 * 512], bf16, kind="ExternalInput")
    # wq_sb[p, 512c + 128m + f] = (wq_core.T)[128c+p, 128m+f]
    wq_d = nc.dram_tensor("wq", [128, DCH * 512], bf16, kind="ExternalInput")
    wk_d = nc.dram_tensor("wk", [128, DCH * 128], bf16, kind="ExternalInput")
    wv_d = nc.dram_tensor("wv", [128, DCH * 128], bf16, kind="ExternalInput")
    # wo_sb[p, 4096h + f] = (wo_core.T)[128h+p, f]
    wo_d = nc.dram_tensor("wo", [128, QH * D], bf16, kind="ExternalInput")
    cosT_d = nc.dram_tensor("cosT", [HD, S], bf16, kind="ExternalInput")
    sinT_d = nc.dram_tensor("sinT", [HD, S], bf16, kind="ExternalInput")
    # small bf16 consts packed into one tensor (one DMA at kernel start):
    # cols 0:128 rmat, 128:256 ident, 256 ones_col, row 0 cols 257:385
    # ones_row
    constsb_d = nc.dram_tensor("constsb", [HD, 385], bf16,
                               kind="ExternalInput")
    masks_d = nc.dram_tensor("masks", [HD, 4 * 512], bf16, kind="ExternalInput")
    constsf_d = nc.dram_tensor("constsf", [HD, 6], f32, kind="ExternalInput")
    onescr_d = nc.dram_tensor("ones_colr", [HD, 1], f32r, kind="ExternalInput")
    # 128KB first-chunk duplicates so the very first matmuls gate on ~0.5MB
    wqc0_d = nc.dram_tensor("wq_c0", [128, 512], bf16, kind="ExternalInput")
    xc0_d = nc.dram_tensor("x_c0", [128, 512], bf16, kind="ExternalInput")
    # out_t[t, n, p, f] = out[128t+p, 512n+f] (bf16; host converts + reduces)
    # out_t[t, np, p, f] = out[128t+p, 1024np+f] (bf16; host converts+reduces)
    out = nc.dram_tensor("out", [NSK, 4, 128, 1024], bf16, kind="ExternalOutput")

    EXP = mybir.ActivationFunctionType.Exp
    LN = mybir.ActivationFunctionType.Ln

    with TileContext(nc) as tc, ExitStack() as ctx:
        const = ctx.enter_context(tc.tile_pool(name="const", bufs=1))
        # weights as quarter tiles so the d-loop can start before the full
        # weight load lands; DMA emission order below is the arrival order.
        wq_sb = [const.tile([128, 4 * 512], bf16, name=f"wq{q}")
                 for q in range(8)]
        wk_sb = [const.tile([128, 8 * 128], bf16, name=f"wk{q}")
                 for q in range(4)]
        wv_sb = [const.tile([128, 8 * 128], bf16, name=f"wv{q}")
                 for q in range(4)]
        constsb = const.tile([HD, 385], bf16)
        masks = const.tile([HD, 4 * 512], bf16)
        constsf = const.tile([HD, 6], f32)
        cosT = const.tile([HD, S], bf16)
        sinT = const.tile([HD, S], bf16)
        wo_sb = const.tile([128, QH * D], bf16)
        rmat = constsb[:, 0:128]
        ident = constsb[:, 128:256]
        ones_col = constsb[:, 256:257]
        ones_row = constsb[0:1, 257:385]
        ksT = constsf[:, 0:5]
        ones_colr_t = const.tile([HD, 1], f32r)
        ones_colr = ones_colr_t[:]

        wq_c0 = const.tile([128, 512], bf16)
        x_c0 = const.tile([128, 512], bf16)
        # minimal start-gating set: first-chunk weights + x (0.5MB), then
        # the rest in order of first use
        nc.sync.dma_start(wq_c0[:], wqc0_d[:, :])
        nc.sync.dma_start(x_c0[:], xc0_d[:, :])
        nc.sync.dma_start(wq_sb[0][:], wq_d[:, 0:4 * 512])
        nc.sync.dma_start(constsb[:], constsb_d[:, :])
        nc.sync.dma_start(constsf[:], constsf_d[:, :])
        nc.sync.dma_start(wk_sb[0][:], wk_d[:, 0:8 * 128])
        nc.sync.dma_start(wv_sb[0][:], wv_d[:, 0:8 * 128])
        nc.sync.dma_start(ones_colr_t[:], onescr_d[:, :])

        persist = ctx.enter_context(tc.tile_pool(name="persist", bufs=1))
        krot = persist.tile([HD, S], bf16)
        vnat = persist.tile([HD, NSK * HD], bf16)  # v tile i at cols i*128

        # PSUM: 8 banks as 4 singles (S0-S3, [128,512]) + 2 doubles
        # (D0/D1, [128,1024] = 2 banks each). Phases time-share by tag:
        #   A(j): accs/rope/vT entirely on S0-S2, so A(j+1) can run on the
        #         PE (as detour work) while B(j)'s exp backlog drains.
        #   B(j<3): scores head-paired on D0; av (both heads) on D1;
        #         dn on DVE; dn-reduce + recip-broadcast on S3.
        #   B(3): no A(4) detour exists, so keep the PE dense instead:
        #         scores depth-2 on D0/D1, av on S0/S1, dn on PE into S2.
        #   C(j): o accumulated in [128,1024] pairs on D0/D1.
        psum = ctx.enter_context(tc.tile_pool(name="psum", bufs=1, space="PSUM"))
        xpool = ctx.enter_context(tc.tile_pool(name="xpool", bufs=10))
        qrotp = ctx.enter_context(tc.tile_pool(name="qrotp", bufs=2))
        attnp = ctx.enter_context(tc.tile_pool(name="attnp", bufs=2))
        stage = ctx.enter_context(tc.tile_pool(name="stage", bufs=2))
        epool = ctx.enter_context(tc.tile_pool(name="epool", bufs=6))
        dnp = ctx.enter_context(tc.tile_pool(name="dnp", bufs=2))
        smallp = ctx.enter_context(tc.tile_pool(name="smallp", bufs=2))
        opool = ctx.enter_context(tc.tile_pool(name="opool", bufs=3))

        def pair2(tile, c0, c1):
            # [128, (2, c1-c0)] view over the two 512-wide halves of a
            # [128, 1024] tile
            return tile[:].rearrange("p (s w) -> p s w", s=2)[:, :, c0:c1]

        for j in range(J):
            sq = slice(j * 512, (j + 1) * 512)

            # ---------------- Phase A(j): projections + RoPE + V -----------
            xq = [xpool.tile([128, 4 * 512], bf16, tag="x", name=f"x{e}")
                  for e in range(XE)]
            if j == 0:
                # ladder: interleave x / wq eighths in consumption order so
                # the cold d-loop advances with DMA arrival; then the rest.
                nc.sync.dma_start(xq[0][:], xh_d[j, 0])
                for e in range(1, XE):
                    nc.sync.dma_start(wq_sb[e][:],
                                      wq_d[:, e * 2048:(e + 1) * 2048])
                    if e % 2 == 0:
                        qtr = e // 2
                        nc.sync.dma_start(
                            wk_sb[qtr][:],
                            wk_d[:, qtr * 1024:(qtr + 1) * 1024])
                        nc.sync.dma_start(
                            wv_sb[qtr][:],
                            wv_d[:, qtr * 1024:(qtr + 1) * 1024])
                    nc.sync.dma_start(xq[e][:], xh_d[j, e])
                nc.sync.dma_start(cosT[:], cosT_d[:, :])
                nc.sync.dma_start(sinT[:], sinT_d[:, :])
                nc.sync.dma_start(masks[:], masks_d[:, :])
                nc.sync.dma_start(wo_sb[:], wo_d[:, :])
            else:
                for e in range(XE):
                    nc.gpsimd.dma_start(xq[e][:], xh_d[j, e])

            # Phase A(j): two d-passes (q0-q2, then q3/k/v) on S0-S2 only
            qrot_j = []
            v_sb = None
            for half in range(2):
                accs = [psum.tile([128, 512], f32, tag=f"S{a}",
                                  name=f"acc{half}_{a}") for a in range(3)]
                for c in range(DCH):
                    cq, c8 = c // 8, c % 8
                    ce, c4 = c // 4, c % 4
                    first = (j == 0 and c == 0)
                    xt = x_c0[:] if first \
                        else xq[ce][:, c4 * 512:(c4 + 1) * 512]
                    wqt = wq_c0 if first else wq_sb[ce]
                    wq_off = 0 if first else c4 * 512
                    st = (c == 0)
                    sp = (c == DCH - 1)
                    if half == 0:
                        for m in range(3):
                            nc.tensor.matmul(
                                accs[m][:],
                                wqt[:, wq_off + m * 128:
                                    wq_off + (m + 1) * 128],
                                xt, start=st, stop=sp)
                    else:
                        nc.tensor.matmul(
                            accs[0][:],
                            wqt[:, wq_off + 3 * 128:wq_off + 4 * 128],
                            xt, start=st, stop=sp)
                        nc.tensor.matmul(accs[1][:],
                                         wk_sb[cq][:, c8 * 128:(c8 + 1) * 128],
                                         xt, start=st, stop=sp)
                        nc.tensor.matmul(accs[2][:],
                                         wv_sb[cq][:, c8 * 128:(c8 + 1) * 128],
                                         xt, start=st, stop=sp)

                # RoPE (q0-q2 after half 0; q3,k after half 1; v copy only)
                srcs = [0, 1, 2] if half == 0 else [3, 4]
                for m in srcs:
                    a = m if half == 0 else m - 3
                    q_sb = stage.tile([128, 512], bf16, tag="qsb")
                    nc.scalar.copy(q_sb[:], accs[a][:])
                    rot_ps = psum.tile([128, 512], f32, tag=f"S{m % 3}",
                                       name=f"rot{m}")
                    nc.tensor.matmul(rot_ps[:], rmat, q_sb[:],
                                     start=True, stop=True)
                    t1 = stage.tile([128, 512], bf16, tag="t1")
                    nc.vector.tensor_mul(t1[:], q_sb[:], cosT[:, sq])
                    t2 = stage.tile([128, 512], bf16, tag="t2")
                    nc.vector.tensor_mul(t2[:], rot_ps[:], sinT[:, sq])
                    if m < QH:
                        dst = qrotp.tile([128, 512], bf16, tag=f"q{m}",
                                         name=f"qr{m}")
                        qrot_j.append(dst)
                        nc.vector.tensor_add(dst[:], t1[:], t2[:])
                    else:
                        nc.vector.tensor_add(krot[:, sq], t1[:], t2[:])
                if half == 1:
                    v_sb = stage.tile([128, 512], bf16, tag="vsb")
                    nc.scalar.copy(v_sb[:], accs[2][:])

            # V transpose into vnat (natural [sk, hd] layout)
            for b in range(4):
                i = 4 * j + b
                vt_ps = psum.tile([128, 512], bf16, tag=f"S{(2 + b) % 3}",
                                  name=f"vt{b}")
                nc.tensor.transpose(vt_ps[:, 0:128],
                                    v_sb[:, b * 128:(b + 1) * 128], ident)
                nc.vector.tensor_copy(vnat[:, i * 128:(i + 1) * 128],
                                      vt_ps[:, 0:128])

            # ---------------- Phase B(j): attention, head-paired ----------
            attn_j = {}
            ni = 4 * j + 4            # sk tiles 0..4j+3 are live
            last = J - 1

            def b_tile(i, h0, h1, s2, e2, av0, av1, dn_acc2):
                """score + exp(+scale) + mask + dn + av for sk tile i of the
                head pair (h0,h1); the two heads land in the two halves of
                s2/e2."""
                delta = i - 4 * j
                # causal trim: diagonal tile delta needs cols >= 128*delta
                c0 = 128 * delta if delta > 0 else 0
                cs0 = slice(c0, 512)
                cs1 = slice(512 + c0, 1024)
                ki = krot[:, i * 128:(i + 1) * 128]
                nc.tensor.matmul(s2[:, cs0], ki, qrot_j[h0][:, cs0],
                                 start=True, stop=True)
                nc.tensor.matmul(s2[:, cs1], ki, qrot_j[h1][:, cs0],
                                 start=True, stop=True)
                # enhance/suppress folded into exp's per-partition scale;
                # one 2D-AP activation covers both heads' halves
                if i < 5 and j >= 1:
                    if j == 1:
                        cb = BOUND - 512        # 99
                        nc.scalar.activation(pair2(e2, 0, cb),
                                             pair2(s2, 0, cb), EXP)
                        nc.scalar.activation(pair2(e2, cb, 512),
                                             pair2(s2, cb, 512), EXP,
                                             scale=ksT[:, i:i + 1])
                    else:
                        nc.scalar.activation(pair2(e2, c0, 512),
                                             pair2(s2, c0, 512), EXP,
                                             scale=ksT[:, i:i + 1])
                else:
                    nc.scalar.activation(pair2(e2, c0, 512),
                                         pair2(s2, c0, 512), EXP)
                if delta >= 0:
                    mki = masks[:, delta * 512 + c0:(delta + 1) * 512]
                    nc.vector.tensor_mul(e2[:, cs0], e2[:, cs0], mki)
                    nc.vector.tensor_mul(e2[:, cs1], e2[:, cs1], mki)
                st = (i == 0)
                sp = (i == ni - 1)
                # denominator accumulates on DVE (off the PE)
                if i == 0:
                    nc.vector.tensor_copy(dn_acc2[:], e2[:])
                else:
                    nc.vector.tensor_add(pair2(dn_acc2, c0, 512),
                                         pair2(dn_acc2, c0, 512),
                                         pair2(e2, c0, 512))
                vi = vnat[:, i * 128:(i + 1) * 128]
                nc.tensor.matmul(av0[:, cs0], vi, e2[:, cs0],
                                 start=st, stop=sp)
                nc.tensor.matmul(av1[:, cs0 if av1 is not av0 else cs1], vi,
                                 e2[:, cs1], start=st, stop=sp)

            def b_tail(h0, h1, dn_acc2, av_of, rbtag):
                """dn partition-reduce, 1/x = exp(-ln(x)), K=1 broadcast,
                and the normalize-multiply for the pair (h0,h1)."""
                recs = []
                for hh in range(2):
                    dnr = psum.tile([1, 512], f32, tag=rbtag, name="dnr")
                    nc.tensor.matmul(dnr[:], ones_colr,
                                     dn_acc2[:, hh * 512:(hh + 1) * 512],
                                     start=True, stop=True)
                    lrec = smallp.tile([1, 512], f32, tag="lrec")
                    nc.scalar.activation(lrec[:], dnr[:], LN)
                    rec2 = smallp.tile([1, 512], bf16, tag="rec2")
                    nc.scalar.activation(rec2[:], lrec[:], EXP, scale=-1.0)
                    recs.append(rec2)
                for hh, h in enumerate((h0, h1)):
                    rb_ps = psum.tile([128, 512], f32, tag=rbtag, name="rb")
                    nc.tensor.matmul(rb_ps[:], ones_row,
                                     recs[hh][:], start=True, stop=True)
                    rb_sb = stage.tile([128, 512], f32, tag="rb")
                    nc.vector.tensor_copy(rb_sb[:], rb_ps[:])
                    at = attnp.tile([128, 512], bf16, tag=f"at{h}",
                                    name=f"at{h}")
                    nc.vector.tensor_mul(at[:], av_of(hh), rb_sb[:])
                    attn_j[h] = at

            if j < last:
                # head pairs sequentially; the PE detours into A(j+1)
                # whenever the exp backlog stalls this pair's pipeline
                for h0 in (0, 2):
                    av2 = psum.tile([128, 1024], f32, tag="D1", name="av2")
                    dn_acc2 = dnp.tile([128, 1024], f32r, tag="dn")
                    for i in range(ni):
                        s2 = psum.tile([128, 1024], f32, tag="D0", name="s2")
                        e2 = epool.tile([128, 1024], bf16, tag="e")
                        b_tile(i, h0, h0 + 1, s2, e2, av2, av2, dn_acc2)
                    b_tail(h0, h0 + 1, dn_acc2,
                           lambda hh, a=av2: a[:, hh * 512:(hh + 1) * 512],
                           "S3")
            else:
                # no A(4) exists to detour into: run BOTH head pairs as
                # independent interleaved pipelines (D0/D1 score chains,
                # av on all four single banks) to keep the PE dense
                av4 = [psum.tile([128, 512], f32, tag=f"S{k}", name=f"av{k}")
                       for k in range(4)]
                dn_accs = [dnp.tile([128, 1024], f32r, tag="dn",
                                    name=f"dna{p}") for p in range(2)]
                for i in range(ni):
                    for p, h0 in enumerate((0, 2)):
                        s2 = psum.tile([128, 1024], f32, tag=f"D{p}",
                                       name="s2")
                        e2 = epool.tile([128, 1024], bf16, tag="e")
                        b_tile(i, h0, h0 + 1, s2, e2,
                               av4[2 * p], av4[2 * p + 1], dn_accs[p])
                for p, h0 in enumerate((0, 2)):
                    b_tail(h0, h0 + 1, dn_accs[p],
                           lambda hh, p=p: av4[2 * p + hh][:], f"D{p}")

            # ---------------- Phase C(j): o_proj in n-pairs ---------------
            for t4 in range(4):
                t = 4 * j + t4
                ts_ = slice(t4 * 128, (t4 + 1) * 128)
                for np_ in range(4):
                    o2 = psum.tile([128, 1024], f32,
                                   tag=f"D{np_ % 2}", name="o2")
                    for h in range(QH):
                        for half in range(2):
                            n = 2 * np_ + half
                            nc.tensor.matmul(
                                o2[:, half * 512:(half + 1) * 512],
                                attn_j[h][:, ts_],
                                wo_sb[:, h * D + n * 512:h * D + (n + 1) * 512],
                                start=(h == 0), stop=(h == QH - 1))
                    o_sb = opool.tile([128, 1024], bf16, tag="osb")
                    nc.vector.tensor_copy(o_sb[:], o2[:])
                    nc.sync.dma_start(out[t, np_], o_sb[:])

    # Split multi-wait instructions (self-loading f32r matmuls allow only
    # one sync wait) onto standalone EventSemaphore instructions.
    import bass_rust
    bass_rust.generate_event_semaphores(nc)
    return nc


def _get_compiled():
    if "nc" not in _CACHE:
        _CACHE["nc"] = _build_bass()
        _CACHE["const"] = _host_constants()
    return _CACHE["nc"], _CACHE["const"]


def kernel(hidden_states, wq, wk, wv, wo, _trace=False):
    import ml_dtypes
    from concourse.bass_utils import run_bass_kernel_spmd

    bf16 = ml_dtypes.bfloat16
    nc, cst = _get_compiled()

    x = np.asarray(hidden_states, dtype=np.float32).reshape(S, D)
    xT = np.ascontiguousarray(x.T)                       # [D, S]
    # xh[j, q, p, 512*c8 + f] = xT[128*(8q+c8)+p, 512j+f]
    xh = np.ascontiguousarray(
        xT.reshape(XE, 4, 128, J, 512).transpose(3, 0, 2, 1, 4)
        .reshape(J, XE, 128, 4 * 512)).astype(bf16)
    wq = np.asarray(wq, dtype=np.float32)
    wk = np.asarray(wk, dtype=np.float32)
    wv = np.asarray(wv, dtype=np.float32)
    wo = np.asarray(wo, dtype=np.float32)
    scale = 1.0 / math.sqrt(HD)

    in_maps = []
    for d in range(NCORES):
        wq_d = wq[d * QH * HD:(d + 1) * QH * HD] * scale      # [512, D]
        wqT = np.ascontiguousarray(wq_d.T)                     # [D, 512]
        wq_sb = (wqT.reshape(DCH, 128, QH, 128).transpose(1, 0, 2, 3)
                 .reshape(128, DCH * 512)).astype(bf16)
        wkT = wk[d * HD:(d + 1) * HD].T                        # [D, 128]
        wk_sb = (wkT.reshape(DCH, 128, 128).transpose(1, 0, 2)
                 .reshape(128, DCH * 128)).astype(bf16)
        wvT = wv[d * HD:(d + 1) * HD].T
        wv_sb = (wvT.reshape(DCH, 128, 128).transpose(1, 0, 2)
                 .reshape(128, DCH * 128)).astype(bf16)
        woT = wo[:, d * QH * HD:(d + 1) * QH * HD].T           # [512, D]
        wo_sb = (woT.reshape(QH, 128, D).transpose(1, 0, 2)
                 .reshape(128, QH * D)).astype(bf16)
        in_maps.append({
            "xh": xh,
            "wq_c0": np.ascontiguousarray(wq_sb[:, 0:512]),
            "x_c0": np.ascontiguousarray(xh[0, 0][:, 0:512]),
            "wq": np.ascontiguousarray(wq_sb),
            "wk": np.ascontiguousarray(wk_sb),
            "wv": np.ascontiguousarray(wv_sb),
            "wo": np.ascontiguousarray(wo_sb),
            "cosT": cst["cosT"], "sinT": cst["sinT"],
            "masks": cst["masks"], "constsb": cst["constsb"],
            "constsf": cst["constsf"], "ones_colr": cst["ones_colr"],
        })

    res = run_bass_kernel_spmd(nc, in_maps, core_ids=list(range(NCORES)),
                               trace=_trace)
    acc = res.results[0]["out"].astype(np.float32)
    for d in range(1, NCORES):
        acc += res.results[d]["out"].astype(np.float32)
    # out_t[t, n, p, f] -> out[128t+p, 512n+f]
    outp = acc.transpose(0, 2, 1, 3).reshape(S, D).astype(np.float32)
    outp = outp.reshape(1, S, D)
    if _trace:
        _CACHE["last_results"] = res
    return outp


# revision 38
# speedup vs baseline: 1.0025x; 1.0025x over previous
"""Trainium2 Bass kernel for nn_AttnAdapter: GQA attention with RoPE,
region-based enhance/suppress score scaling, causal mask, o_proj.

Sharding: tensor-parallel over heads across 8 NeuronCores. Core d holds
q-heads 4d..4d+3 (wq rows), kv-head d (wk/wv rows), and wo columns
512d..512(d+1). Each core computes a full [S, D] partial of the output;
the host sums the 8 partials (the TP all-reduce, done at unshard time).

Structure (per core, ~420us HW vs 791us baseline):
- bf16 operands everywhere on-chip (fp32 PSUM accumulation); weights and
  x are SBUF-resident, loaded exactly once, with a laddered DMA issue
  order so the first matmuls start ~12us into the kernel.
- One fused loop over the four 512-row q blocks j: projections+RoPE ->
  attention -> o_proj, so exp/evacuations/DMA always overlap dense PE
  work from a neighboring phase.
- PSUM's 8 banks are partitioned by tag (4 singles S0-S3 + 2 doubles
  D0/D1) such that phase A(j+1) (on S0-S2) is always schedulable while
  B(j)'s exp backlog drains (on D0/D1) and C(j) evacuates: the PE
  "detours" into next-block projections instead of idling on softmax.
- Attention is head-paired: one [128,1024] score tile and one 2D-AP exp
  activation cover two heads per sk tile (the enhance/suppress scale is
  per key position, folded into exp's per-partition scale operand);
  causal-diagonal tiles are column-trimmed. The softmax denominator
  accumulates on the vector engine, off the PE. For j=3 (no next block
  to detour into) both head pairs run as interleaved pipelines.
- o_proj accumulates [128,1024] n-pairs, evacuated once per pair on the
  vector engine; the host sums the 8 cores' bf16 partials (the TP
  all-reduce at unshard time).
"""

import math

import numpy as np

# ---- problem constants (hardcoded; kernel.py must be self-contained) ----
S = 2048          # sequence length
D = 4096          # model dim
HD = 128          # head dim
NCORES = 8
QH = 4            # q heads per core
SYS_LEN, IMG_LEN = 35, 576
BOUND = SYS_LEN + IMG_LEN          # 611
ENH, SUP = 1.5, 0.5
ROPE_BASE = 10000.0

J = 4             # sq tiles of 512
NSK = 16          # sk tiles of 128
DCH = 32          # D chunks of 128
XE = 8            # x eighth-blocks per j (4 d-chunks each)

_CACHE = {}


def _host_constants():
    import ml_dtypes
    bf16 = ml_dtypes.bfloat16

    inv_freq = 1.0 / (ROPE_BASE ** (np.arange(0, HD, 2, dtype=np.float32) / HD))
    pos = np.arange(S, dtype=np.float32)
    freqs = pos[:, None] * inv_freq[None, :]              # [S, 64]
    emb = np.concatenate([freqs, freqs], axis=-1)         # [S, 128]
    cosT = np.ascontiguousarray(np.cos(emb).T).astype(bf16)  # [128, S]
    sinT = np.ascontiguousarray(np.sin(emb).T).astype(bf16)

    # rotate_half as a matmul: rot = R @ q (in [hd, s] layout).
    # matmul(out, lhsT, rhs) = lhsT.T @ rhs, so feed RT = R.T.
    RT = np.zeros((HD, HD), dtype=np.float32)
    half = HD // 2
    for c in range(half):
        RT[c + half, c] = -1.0      # rot[c] = -q[c+64]
    for c in range(half, HD):
        RT[c - half, c] = 1.0       # rot[c] = q[c-64]

    ident = np.eye(HD, dtype=np.float32).astype(bf16)

    # Diagonal-tile causal masks, T layout [sk 128, sq 512]:
    # tile (i=4j+delta, j): valid (keep) iff sq >= sk  <=>  f >= 128*delta + p
    masks = np.zeros((HD, 4 * 512), dtype=np.float32)
    p = np.arange(128)[:, None]
    f = np.arange(512)[None, :]
    for delta in range(4):
        masks[:, delta * 512:(delta + 1) * 512] = (f >= 128 * delta + p)
    maskneg = ((1.0 - masks.astype(np.float32)) * -1e30).astype(bf16)
    masks = masks.astype(bf16)

    # key_scale in partition layout per sk-tile: ksT[p, i] = scale(128*i+p)
    kpos = np.arange(S)
    key_scale = np.where(kpos < SYS_LEN, SUP,
                         np.where(kpos < BOUND, ENH, 1.0)).astype(np.float32)
    ksT = np.ascontiguousarray(key_scale[:5 * 128].reshape(5, 128).T)  # [128, 5]

    constsb = np.zeros((HD, 385), dtype=bf16)
    constsb[:, 0:128] = RT.astype(bf16)
    constsb[:, 128:256] = ident
    constsb[:, 256] = np.ones(HD, dtype=np.float32).astype(bf16)
    constsb[0, 257:385] = np.ones(HD, dtype=np.float32).astype(bf16)
    constsf = np.ones((HD, 6), dtype=np.float32)
    constsf[:, 0:5] = ksT
    return dict(cosT=cosT, sinT=sinT, masks=masks, maskneg=maskneg,
                constsb=constsb, constsf=constsf,
                ones_colr=np.ones((HD, 1), dtype=np.float32))


def _build_bass():
    import concourse.bass as bass
    import concourse.mybir as mybir
    from concourse.tile import TileContext
    from contextlib import ExitStack

    f32 = mybir.dt.float32
    f32r = mybir.dt.float32r
    bf16 = mybir.dt.bfloat16

    nc = bass.Bass()
    # x eighths: xh[j, e, p, 512*c4 + f] = x.T[128*(4e+c4)+p, 512j+f]
    xh_d = nc.dram_tensor("xh", [J, XE, 128, 4 * 512], bf16, kind="ExternalInput")
    # wq_sb[p, 512c + 128m + f] = (wq_core.T)[128c+p, 128m+f]
    wq_d = nc.dram_tensor("wq", [128, DCH * 512], bf16, kind="ExternalInput")
    wk_d = nc.dram_tensor("wk", [128, DCH * 128], bf16, kind="ExternalInput")
    wv_d = nc.dram_tensor("wv", [128, DCH * 128], bf16, kind="ExternalInput")
    # wo_sb[p, 4096h + f] = (wo_core.T)[128h+p, f]
    wo_d = nc.dram_tensor("wo", [128, QH * D], bf16, kind="ExternalInput")
    cosT_d = nc.dram_tensor("cosT", [HD, S], bf16, kind="ExternalInput")
    sinT_d = nc.dram_tensor("sinT", [HD, S], bf16, kind="ExternalInput")
    # small bf16 consts packed into one tensor (one DMA at kernel start):
    # cols 0:128 rmat, 128:256 ident, 256 ones_col, row 0 cols 257:385
    # ones_row
    constsb_d = nc.dram_tensor("constsb", [HD, 385], bf16,
                               kind="ExternalInput")
    masks_d = nc.dram_tensor("masks", [HD, 4 * 512], bf16, kind="ExternalInput")
    maskneg_d = nc.dram_tensor("maskneg", [HD, 4 * 512], bf16,
                               kind="ExternalInput")
    constsf_d = nc.dram_tensor("constsf", [HD, 6], f32, kind="ExternalInput")
    onescr_d = nc.dram_tensor("ones_colr", [HD, 1], f32r, kind="ExternalInput")
    # 128KB first-chunk duplicates so the very first matmuls gate on ~0.5MB
    wqc0_d = nc.dram_tensor("wq_c0", [128, 512], bf16, kind="ExternalInput")
    xc0_d = nc.dram_tensor("x_c0", [128, 512], bf16, kind="ExternalInput")
    # out_t[t, n, p, f] = out[128t+p, 512n+f] (bf16; host converts + reduces)
    # out_t[t, np, p, f] = out[128t+p, 1024np+f] (bf16; host converts+reduces)
    out = nc.dram_tensor("out", [NSK, 4, 128, 1024], bf16, kind="ExternalOutput")

    EXP = mybir.ActivationFunctionType.Exp
    LN = mybir.ActivationFunctionType.Ln

    with TileContext(nc) as tc, ExitStack() as ctx:
        const = ctx.enter_context(tc.tile_pool(name="const", bufs=1))
        # weights as quarter tiles so the d-loop can start before the full
        # weight load lands; DMA emission order below is the arrival order.
        wq_sb = [const.tile([128, 4 * 512], bf16, name=f"wq{q}")
                 for q in range(8)]
        wk_sb = [const.tile([128, 8 * 128], bf16, name=f"wk{q}")
                 for q in range(4)]
        wv_sb = [const.tile([128, 8 * 128], bf16, name=f"wv{q}")
                 for q in range(4)]
        constsb = const.tile([HD, 385], bf16)
        masks = const.tile([HD, 4 * 512], bf16)
        maskneg = const.tile([HD, 4 * 512], bf16)
        constsf = const.tile([HD, 6], f32)
        cosT = const.tile([HD, S], bf16)
        sinT = const.tile([HD, S], bf16)
        wo_sb = const.tile([128, QH * D], bf16)
        rmat = constsb[:, 0:128]
        ident = constsb[:, 128:256]
        ones_col = constsb[:, 256:257]
        ones_row = constsb[0:1, 257:385]
        ksT = constsf[:, 0:5]
        ones_colr_t = const.tile([HD, 1], f32r)
        ones_colr = ones_colr_t[:]

        wq_c0 = const.tile([128, 512], bf16)
        x_c0 = const.tile([128, 512], bf16)
        # minimal start-gating set: first-chunk weights + x (0.5MB), then
        # the rest in order of first use
        nc.sync.dma_start(wq_c0[:], wqc0_d[:, :])
        nc.sync.dma_start(x_c0[:], xc0_d[:, :])
        nc.sync.dma_start(wq_sb[0][:], wq_d[:, 0:4 * 512])
        nc.sync.dma_start(constsb[:], constsb_d[:, :])
        nc.sync.dma_start(constsf[:], constsf_d[:, :])
        nc.sync.dma_start(wk_sb[0][:], wk_d[:, 0:8 * 128])
        nc.sync.dma_start(wv_sb[0][:], wv_d[:, 0:8 * 128])
        nc.sync.dma_start(ones_colr_t[:], onescr_d[:, :])

        persist = ctx.enter_context(tc.tile_pool(name="persist", bufs=1))
        krot = persist.tile([HD, S], bf16)
        vnat = persist.tile([HD, NSK * HD], bf16)  # v tile i at cols i*128

        # PSUM: 8 banks as 4 singles (S0-S3, [128,512]) + 2 doubles
        # (D0/D1, [128,1024] = 2 banks each). Phases time-share by tag:
        #   A(j): accs/rope/vT entirely on S0-S2, so A(j+1) can run on the
        #         PE (as detour work) while B(j)'s exp backlog drains.
        #   B(j<3): scores head-paired on D0; av (both heads) on D1;
        #         dn on DVE; dn-reduce + recip-broadcast on S3.
        #   B(3): no A(4) detour exists, so keep the PE dense instead:
        #         scores depth-2 on D0/D1, av on S0/S1, dn on PE into S2.
        #   C(j): o accumulated in [128,1024] pairs on D0/D1.
        psum = ctx.enter_context(tc.tile_pool(name="psum", bufs=1, space="PSUM"))
        xpool = ctx.enter_context(tc.tile_pool(name="xpool", bufs=10))
        qrotp = ctx.enter_context(tc.tile_pool(name="qrotp", bufs=2))
        attnp = ctx.enter_context(tc.tile_pool(name="attnp", bufs=2))
        stage = ctx.enter_context(tc.tile_pool(name="stage", bufs=2))
        epool = ctx.enter_context(tc.tile_pool(name="epool", bufs=6))
        dnp = ctx.enter_context(tc.tile_pool(name="dnp", bufs=2))
        smallp = ctx.enter_context(tc.tile_pool(name="smallp", bufs=2))
        opool = ctx.enter_context(tc.tile_pool(name="opool", bufs=3))

        def pair2(tile, c0, c1):
            # [128, (2, c1-c0)] view over the two 512-wide halves of a
            # [128, 1024] tile
            return tile[:].rearrange("p (s w) -> p s w", s=2)[:, :, c0:c1]

        for j in range(J):
            sq = slice(j * 512, (j + 1) * 512)

            # ---------------- Phase A(j): projections + RoPE + V -----------
            xq = [xpool.tile([128, 4 * 512], bf16, tag="x", name=f"x{e}")
                  for e in range(XE)]
            if j == 0:
                # ladder: interleave x / wq eighths in consumption order so
                # the cold d-loop advances with DMA arrival; then the rest.
                nc.sync.dma_start(xq[0][:], xh_d[j, 0])
                for e in range(1, XE):
                    nc.sync.dma_start(wq_sb[e][:],
                                      wq_d[:, e * 2048:(e + 1) * 2048])
                    if e % 2 == 0:
                        qtr = e // 2
                        nc.sync.dma_start(
                            wk_sb[qtr][:],
                            wk_d[:, qtr * 1024:(qtr + 1) * 1024])
                        nc.sync.dma_start(
                            wv_sb[qtr][:],
                            wv_d[:, qtr * 1024:(qtr + 1) * 1024])
                    nc.sync.dma_start(xq[e][:], xh_d[j, e])
                nc.sync.dma_start(cosT[:], cosT_d[:, :])
                nc.sync.dma_start(sinT[:], sinT_d[:, :])
                nc.sync.dma_start(masks[:], masks_d[:, :])
                nc.sync.dma_start(maskneg[:], maskneg_d[:, :])
                nc.sync.dma_start(wo_sb[:], wo_d[:, :])
            else:
                for e in range(XE):
                    nc.gpsimd.dma_start(xq[e][:], xh_d[j, e])

            # Phase A(j): two d-passes (q0-q2, then q3/k/v) on S0-S2 only
            qrot_j = []
            v_sb = None
            for half in range(2):
                accs = [psum.tile([128, 512], f32, tag=f"S{a}",
                                  name=f"acc{half}_{a}") for a in range(3)]
                for c in range(DCH):
                    cq, c8 = c // 8, c % 8
                    ce, c4 = c // 4, c % 4
                    first = (j == 0 and c == 0)
                    xt = x_c0[:] if first \
                        else xq[ce][:, c4 * 512:(c4 + 1) * 512]
                    wqt = wq_c0 if first else wq_sb[ce]
                    wq_off = 0 if first else c4 * 512
                    st = (c == 0)
                    sp = (c == DCH - 1)
                    if half == 0:
                        for m in range(3):
                            nc.tensor.matmul(
                                accs[m][:],
                                wqt[:, wq_off + m * 128:
                                    wq_off + (m + 1) * 128],
                                xt, start=st, stop=sp)
                    else:
                        nc.tensor.matmul(
                            accs[0][:],
                            wqt[:, wq_off + 3 * 128:wq_off + 4 * 128],
                            xt, start=st, stop=sp)
                        nc.tensor.matmul(accs[1][:],
                                         wk_sb[cq][:, c8 * 128:(c8 + 1) * 128],
                                         xt, start=st, stop=sp)
                        nc.tensor.matmul(accs[2][:],
                                         wv_sb[cq][:, c8 * 128:(c8 + 1) * 128],
                                         xt, start=st, stop=sp)

                # RoPE (q0-q2 after half 0; q3,k after half 1; v copy only)
                srcs = [0, 1, 2] if half == 0 else [3, 4]
                for m in srcs:
                    a = m if half == 0 else m - 3
                    q_sb = stage.tile([128, 512], bf16, tag="qsb")
                    nc.scalar.copy(q_sb[:], accs[a][:])
                    rot_ps = psum.tile([128, 512], f32, tag=f"S{m % 3}",
                                       name=f"rot{m}")
                    nc.tensor.matmul(rot_ps[:], rmat, q_sb[:],
                                     start=True, stop=True)
                    t1 = stage.tile([128, 512], bf16, tag="t1")
                    nc.vector.tensor_mul(t1[:], q_sb[:], cosT[:, sq])
                    t2 = stage.tile([128, 512], bf16, tag="t2")
                    nc.vector.tensor_mul(t2[:], rot_ps[:], sinT[:, sq])
                    if m < QH:
                        dst = qrotp.tile([128, 512], bf16, tag=f"q{m}",
                                         name=f"qr{m}")
                        qrot_j.append(dst)
                        nc.vector.tensor_add(dst[:], t1[:], t2[:])
                    else:
                        nc.vector.tensor_add(krot[:, sq], t1[:], t2[:])
                if half == 1:
                    v_sb = stage.tile([128, 512], bf16, tag="vsb")
                    nc.scalar.copy(v_sb[:], accs[2][:])

            # V transpose into vnat (natural [sk, hd] layout)
            for b in range(4):
                i = 4 * j + b
                vt_ps = psum.tile([128, 512], bf16, tag=f"S{(2 + b) % 3}",
                                  name=f"vt{b}")
                nc.tensor.transpose(vt_ps[:, 0:128],
                                    v_sb[:, b * 128:(b + 1) * 128], ident)
                nc.vector.tensor_copy(vnat[:, i * 128:(i + 1) * 128],
                                      vt_ps[:, 0:128])

            # ---------------- Phase B(j): attention, head-paired ----------
            attn_j = {}
            ni = 4 * j + 4            # sk tiles 0..4j+3 are live
            last = J - 1

            def b_tile(i, h0, h1, s2, e2, av0, av1, dn_acc2):
                """score + exp(+scale) + mask + dn + av for sk tile i of the
                head pair (h0,h1); the two heads land in the two halves of
                s2/e2."""
                delta = i - 4 * j
                # causal trim: diagonal tile delta needs cols >= 128*delta
                c0 = 128 * delta if delta > 0 else 0
                cs0 = slice(c0, 512)
                cs1 = slice(512 + c0, 1024)
                ki = krot[:, i * 128:(i + 1) * 128]
                mm_mask = (j == J - 1 and delta >= 0)
                nc.tensor.matmul(s2[:, cs0], ki, qrot_j[h0][:, cs0],
                                 start=True, stop=not mm_mask)
                nc.tensor.matmul(s2[:, cs1], ki, qrot_j[h1][:, cs0],
                                 start=True, stop=not mm_mask)
                if mm_mask:
                    # additive -1e30 causal mask on the PE itself: skips two
                    # cross-engine hops in the only region with no detour
                    mneg = maskneg[:, delta * 512 + c0:(delta + 1) * 512]
                    nc.tensor.matmul(s2[:, cs0], ident, mneg,
                                     start=False, stop=True)
                    nc.tensor.matmul(s2[:, cs1], ident, mneg,
                                     start=False, stop=True)
                # enhance/suppress folded into exp's per-partition scale;
                # one 2D-AP activation covers both heads' halves
                if i < 5 and j >= 1:
                    if j == 1:
                        cb = BOUND - 512        # 99
                        nc.scalar.activation(pair2(e2, 0, cb),
                                             pair2(s2, 0, cb), EXP)
                        nc.scalar.activation(pair2(e2, cb, 512),
                                             pair2(s2, cb, 512), EXP,
                                             scale=ksT[:, i:i + 1])
                    else:
                        nc.scalar.activation(pair2(e2, c0, 512),
                                             pair2(s2, c0, 512), EXP,
                                             scale=ksT[:, i:i + 1])
                else:
                    nc.scalar.activation(pair2(e2, c0, 512),
                                         pair2(s2, c0, 512), EXP)
                if delta >= 0 and not mm_mask:
                    mki = masks[:, delta * 512 + c0:(delta + 1) * 512]
                    nc.vector.tensor_mul(e2[:, cs0], e2[:, cs0], mki)
                    nc.vector.tensor_mul(e2[:, cs1], e2[:, cs1], mki)
                st = (i == 0)
                sp = (i == ni - 1)
                # denominator accumulates off the PE; at j=3 the second
                # head-pair chain uses the otherwise-idle gpsimd engine
                dn_eng = nc.gpsimd if (j == J - 1 and h0 == 2) else nc.vector
                if i == 0:
                    dn_eng.tensor_copy(dn_acc2[:], e2[:])
                else:
                    dn_eng.tensor_add(pair2(dn_acc2, c0, 512),
                                      pair2(dn_acc2, c0, 512),
                                      pair2(e2, c0, 512))
                vi = vnat[:, i * 128:(i + 1) * 128]
                nc.tensor.matmul(av0[:, cs0], vi, e2[:, cs0],
                                 start=st, stop=sp)
                nc.tensor.matmul(av1[:, cs0 if av1 is not av0 else cs1], vi,
                                 e2[:, cs1], start=st, stop=sp)

            def b_tail(h0, h1, dn_acc2, av_of, rbtag):
                """dn partition-reduce, 1/x = exp(-ln(x)), K=1 broadcast,
                and the normalize-multiply for the pair (h0,h1)."""
                recs = []
                for hh in range(2):
                    dnr = psum.tile([1, 512], f32, tag=rbtag, name="dnr")
                    nc.tensor.matmul(dnr[:], ones_colr,
                                     dn_acc2[:, hh * 512:(hh + 1) * 512],
                                     start=True, stop=True)
                    lrec = smallp.tile([1, 512], f32, tag="lrec")
                    nc.scalar.activation(lrec[:], dnr[:], LN)
                    rec2 = smallp.tile([1, 512], bf16, tag="rec2")
                    nc.scalar.activation(rec2[:], lrec[:], EXP, scale=-1.0)
                    recs.append(rec2)
                for hh, h in enumerate((h0, h1)):
                    rb_ps = psum.tile([128, 512], f32, tag=rbtag, name="rb")
                    nc.tensor.matmul(rb_ps[:], ones_row,
                                     recs[hh][:], start=True, stop=True)
                    rb_sb = stage.tile([128, 512], f32, tag="rb")
                    nc.vector.tensor_copy(rb_sb[:], rb_ps[:])
                    at = attnp.tile([128, 512], bf16, tag=f"at{h}",
                                    name=f"at{h}")
                    nc.vector.tensor_mul(at[:], av_of(hh), rb_sb[:])
                    attn_j[h] = at

            if j < last:
                # head pairs sequentially; the PE detours into A(j+1)
                # whenever the exp backlog stalls this pair's pipeline
                for h0 in (0, 2):
                    av2 = psum.tile([128, 1024], f32, tag="D1", name="av2")
                    dn_acc2 = dnp.tile([128, 1024], f32r, tag="dn")
                    for i in range(ni):
                        s2 = psum.tile([128, 1024], f32, tag="D0", name="s2")
                        e2 = epool.tile([128, 1024], bf16, tag="e")
                        b_tile(i, h0, h0 + 1, s2, e2, av2, av2, dn_acc2)
                    b_tail(h0, h0 + 1, dn_acc2,
                           lambda hh, a=av2: a[:, hh * 512:(hh + 1) * 512],
                           "S3")
            else:
                # no A(4) exists to detour into: run BOTH head pairs as
                # independent interleaved pipelines (D0/D1 score chains,
                # av on all four single banks) to keep the PE dense
                av4 = [psum.tile([128, 512], f32, tag=f"S{k}", name=f"av{k}")
                       for k in range(4)]
                dn_accs = [dnp.tile([128, 1024], f32r, tag="dn",
                                    name=f"dna{p}") for p in range(2)]
                for i in range(ni):
                    for p, h0 in enumerate((0, 2)):
                        s2 = psum.tile([128, 1024], f32, tag=f"D{p}",
                                       name="s2")
                        e2 = epool.tile([128, 1024], bf16, tag="e")
                        b_tile(i, h0, h0 + 1, s2, e2,
                               av4[2 * p], av4[2 * p + 1], dn_accs[p])
                for p, h0 in enumerate((0, 2)):
                    b_tail(h0, h0 + 1, dn_accs[p],
                           lambda hh, p=p: av4[2 * p + hh][:], f"D{p}")

            # ---------------- Phase C(j): o_proj in n-pairs ---------------
            for t4 in range(4):
                t = 4 * j + t4
                ts_ = slice(t4 * 128, (t4 + 1) * 128)
                for np_ in range(4):
                    o2 = psum.tile([128, 1024], f32,
                                   tag=f"D{np_ % 2}", name="o2")
                    for h in range(QH):
                        for half in range(2):
                            n = 2 * np_ + half
                            nc.tensor.matmul(
                                o2[:, half * 512:(half + 1) * 512],
                                attn_j[h][:, ts_],
                                wo_sb[:, h * D + n * 512:h * D + (n + 1) * 512],
                                start=(h == 0), stop=(h == QH - 1))
                    o_sb = opool.tile([128, 1024], bf16, tag="osb")
                    nc.vector.tensor_copy(o_sb[:], o2[:])
                    nc.sync.dma_start(out[t, np_], o_sb[:])

    # Split multi-wait instructions (self-loading f32r matmuls allow only
    # one sync wait) onto standalone EventSemaphore instructions.
    import bass_rust
    bass_rust.generate_event_semaphores(nc)
    return nc


def _get_compiled():
    if "nc" not in _CACHE:
        _CACHE["nc"] = _build_bass()
        _CACHE["const"] = _host_constants()
    return _CACHE["nc"], _CACHE["const"]


def kernel(hidden_states, wq, wk, wv, wo, _trace=False):
    import ml_dtypes
    from concourse.bass_utils import run_bass_kernel_spmd

    bf16 = ml_dtypes.bfloat16
    nc, cst = _get_compiled()

    x = np.asarray(hidden_states, dtype=np.float32).reshape(S, D)
    xT = np.ascontiguousarray(x.T)                       # [D, S]
    # xh[j, q, p, 512*c8 + f] = xT[128*(8q+c8)+p, 512j+f]
    xh = np.ascontiguousarray(
        xT.reshape(XE, 4, 128, J, 512).transpose(3, 0, 2, 1, 4)
        .reshape(J, XE, 128, 4 * 512)).astype(bf16)
    wq = np.asarray(wq, dtype=np.float32)
    wk = np.asarray(wk, dtype=np.float32)
    wv = np.asarray(wv, dtype=np.float32)
    wo = np.asarray(wo, dtype=np.float32)
    scale = 1.0 / math.sqrt(HD)

    in_maps = []
    for d in range(NCORES):
        wq_d = wq[d * QH * HD:(d + 1) * QH * HD] * scale      # [512, D]
        wqT = np.ascontiguousarray(wq_d.T)                     # [D, 512]
        wq_sb = (wqT.reshape(DCH, 128, QH, 128).transpose(1, 0, 2, 3)
                 .reshape(128, DCH * 512)).astype(bf16)
        wkT = wk[d * HD:(d + 1) * HD].T                        # [D, 128]
        wk_sb = (wkT.reshape(DCH, 128, 128).transpose(1, 0, 2)
                 .reshape(128, DCH * 128)).astype(bf16)
        wvT = wv[d * HD:(d + 1) * HD].T
        wv_sb = (wvT.reshape(DCH, 128, 128).transpose(1, 0, 2)
                 .reshape(128, DCH * 128)).astype(bf16)
        woT = wo[:, d * QH * HD:(d + 1) * QH * HD].T           # [512, D]
        wo_sb = (woT.reshape(QH, 128, D).transpose(1, 0, 2)
                 .reshape(128, QH * D)).astype(bf16)
        in_maps.append({
            "xh": xh,
            "wq_c0": np.ascontiguousarray(wq_sb[:, 0:512]),
            "x_c0": np.ascontiguousarray(xh[0, 0][:, 0:512]),
            "wq": np.ascontiguousarray(wq_sb),
            "wk": np.ascontiguousarray(wk_sb),
            "wv": np.ascontiguousarray(wv_sb),
            "wo": np.ascontiguousarray(wo_sb),
            "cosT": cst["cosT"], "sinT": cst["sinT"],
            "masks": cst["masks"], "maskneg": cst["maskneg"],
            "constsb": cst["constsb"],
            "constsf": cst["constsf"], "ones_colr": cst["ones_colr"],
        })

    res = run_bass_kernel_spmd(nc, in_maps, core_ids=list(range(NCORES)),
                               trace=_trace)
    acc = res.results[0]["out"].astype(np.float32)
    for d in range(1, NCORES):
        acc += res.results[d]["out"].astype(np.float32)
    # out_t[t, n, p, f] -> out[128t+p, 512n+f]
    outp = acc.transpose(0, 2, 1, 3).reshape(S, D).astype(np.float32)
    outp = outp.reshape(1, S, D)
    if _trace:
        _CACHE["last_results"] = res
    return outp


# revision 39
# speedup vs baseline: 1.0030x; 1.0005x over previous
"""Trainium2 Bass kernel for nn_AttnAdapter: GQA attention with RoPE,
region-based enhance/suppress score scaling, causal mask, o_proj.

Sharding: tensor-parallel over heads across 8 NeuronCores. Core d holds
q-heads 4d..4d+3 (wq rows), kv-head d (wk/wv rows), and wo columns
512d..512(d+1). Each core computes a full [S, D] partial of the output;
the host sums the 8 partials (the TP all-reduce, done at unshard time).

Structure (per core, ~420us HW vs 791us baseline):
- bf16 operands everywhere on-chip (fp32 PSUM accumulation); weights and
  x are SBUF-resident, loaded exactly once, with a laddered DMA issue
  order so the first matmuls start ~12us into the kernel.
- One fused loop over the four 512-row q blocks j: projections+RoPE ->
  attention -> o_proj, so exp/evacuations/DMA always overlap dense PE
  work from a neighboring phase.
- PSUM's 8 banks are partitioned by tag (4 singles S0-S3 + 2 doubles
  D0/D1) such that phase A(j+1) (on S0-S2) is always schedulable while
  B(j)'s exp backlog drains (on D0/D1) and C(j) evacuates: the PE
  "detours" into next-block projections instead of idling on softmax.
- Attention is head-paired: one [128,1024] score tile and one 2D-AP exp
  activation cover two heads per sk tile (the enhance/suppress scale is
  per key position, folded into exp's per-partition scale operand);
  causal-diagonal tiles are column-trimmed. The softmax denominator
  accumulates on the vector engine, off the PE. For j=3 (no next block
  to detour into) both head pairs run as interleaved pipelines.
- o_proj accumulates [128,1024] n-pairs, evacuated once per pair on the
  vector engine; the host sums the 8 cores' bf16 partials (the TP
  all-reduce at unshard time).
"""

import math

import numpy as np

# ---- problem constants (hardcoded; kernel.py must be self-contained) ----
S = 2048          # sequence length
D = 4096          # model dim
HD = 128          # head dim
NCORES = 8
QH = 4            # q heads per core
SYS_LEN, IMG_LEN = 35, 576
BOUND = SYS_LEN + IMG_LEN          # 611
ENH, SUP = 1.5, 0.5
ROPE_BASE = 10000.0

J = 4             # sq tiles of 512
NSK = 16          # sk tiles of 128
DCH = 32          # D chunks of 128
XE = 8            # x eighth-blocks per j (4 d-chunks each)

_CACHE = {}


def _host_constants():
    import ml_dtypes
    bf16 = ml_dtypes.bfloat16

    inv_freq = 1.0 / (ROPE_BASE ** (np.arange(0, HD, 2, dtype=np.float32) / HD))
    pos = np.arange(S, dtype=np.float32)
    freqs = pos[:, None] * inv_freq[None, :]              # [S, 64]
    emb = np.concatenate([freqs, freqs], axis=-1)         # [S, 128]
    cosT = np.ascontiguousarray(np.cos(emb).T).astype(bf16)  # [128, S]
    sinT = np.ascontiguousarray(np.sin(emb).T).astype(bf16)

    # rotate_half as a matmul: rot = R @ q (in [hd, s] layout).
    # matmul(out, lhsT, rhs) = lhsT.T @ rhs, so feed RT = R.T.
    RT = np.zeros((HD, HD), dtype=np.float32)
    half = HD // 2
    for c in range(half):
        RT[c + half, c] = -1.0      # rot[c] = -q[c+64]
    for c in range(half, HD):
        RT[c - half, c] = 1.0       # rot[c] = q[c-64]

    ident = np.eye(HD, dtype=np.float32).astype(bf16)

    # Diagonal-tile causal masks, T layout [sk 128, sq 512]:
    # tile (i=4j+delta, j): valid (keep) iff sq >= sk  <=>  f >= 128*delta + p
    masks = np.zeros((HD, 4 * 512), dtype=np.float32)
    p = np.arange(128)[:, None]
    f = np.arange(512)[None, :]
    for delta in range(4):
        masks[:, delta * 512:(delta + 1) * 512] = (f >= 128 * delta + p)
    maskneg = ((1.0 - masks.astype(np.float32)) * -1e30).astype(bf16)
    masks = masks.astype(bf16)

    # key_scale in partition layout per sk-tile: ksT[p, i] = scale(128*i+p)
    kpos = np.arange(S)
    key_scale = np.where(kpos < SYS_LEN, SUP,
                         np.where(kpos < BOUND, ENH, 1.0)).astype(np.float32)
    ksT = np.ascontiguousarray(key_scale[:5 * 128].reshape(5, 128).T)  # [128, 5]

    constsb = np.zeros((HD, 385), dtype=bf16)
    constsb[:, 0:128] = RT.astype(bf16)
    constsb[:, 128:256] = ident
    constsb[:, 256] = np.ones(HD, dtype=np.float32).astype(bf16)
    constsb[0, 257:385] = np.ones(HD, dtype=np.float32).astype(bf16)
    constsf = np.ones((HD, 6), dtype=np.float32)
    constsf[:, 0:5] = ksT
    return dict(cosT=cosT, sinT=sinT, masks=masks, maskneg=maskneg,
                constsb=constsb, constsf=constsf,
                ones_colr=np.ones((HD, 1), dtype=np.float32))


def _build_bass():
    import concourse.bass as bass
    import concourse.mybir as mybir
    from concourse.tile import TileContext
    from contextlib import ExitStack

    f32 = mybir.dt.float32
    f32r = mybir.dt.float32r
    bf16 = mybir.dt.bfloat16

    nc = bass.Bass()
    # x eighths: xh[j, e, p, 512*c4 + f] = x.T[128*(4e+c4)+p, 512j+f]
    xh_d = nc.dram_tensor("xh", [J, XE, 128, 4 * 512], bf16, kind="ExternalInput")
    # wq_sb[p, 512c + 128m + f] = (wq_core.T)[128c+p, 128m+f]
    wq_d = nc.dram_tensor("wq", [128, DCH * 512], bf16, kind="ExternalInput")
    wk_d = nc.dram_tensor("wk", [128, DCH * 128], bf16, kind="ExternalInput")
    wv_d = nc.dram_tensor("wv", [128, DCH * 128], bf16, kind="ExternalInput")
    # wo_sb[p, 4096h + f] = (wo_core.T)[128h+p, f]
    wo_d = nc.dram_tensor("wo", [128, QH * D], bf16, kind="ExternalInput")
    cosT_d = nc.dram_tensor("cosT", [HD, S], bf16, kind="ExternalInput")
    sinT_d = nc.dram_tensor("sinT", [HD, S], bf16, kind="ExternalInput")
    # small bf16 consts packed into one tensor (one DMA at kernel start):
    # cols 0:128 rmat, 128:256 ident, 256 ones_col, row 0 cols 257:385
    # ones_row
    constsb_d = nc.dram_tensor("constsb", [HD, 385], bf16,
                               kind="ExternalInput")
    masks_d = nc.dram_tensor("masks", [HD, 4 * 512], bf16, kind="ExternalInput")
    maskneg_d = nc.dram_tensor("maskneg", [HD, 4 * 512], bf16,
                               kind="ExternalInput")
    constsf_d = nc.dram_tensor("constsf", [HD, 6], f32, kind="ExternalInput")
    onescr_d = nc.dram_tensor("ones_colr", [HD, 1], f32r, kind="ExternalInput")
    # 128KB first-chunk duplicates so the very first matmuls gate on ~0.5MB
    wqc0_d = nc.dram_tensor("wq_c0", [128, 512], bf16, kind="ExternalInput")
    xc0_d = nc.dram_tensor("x_c0", [128, 512], bf16, kind="ExternalInput")
    # out_t[t, n, p, f] = out[128t+p, 512n+f] (bf16; host converts + reduces)
    # out_t[t, np, p, f] = out[128t+p, 1024np+f] (bf16; host converts+reduces)
    out = nc.dram_tensor("out", [NSK, 4, 128, 1024], bf16, kind="ExternalOutput")

    EXP = mybir.ActivationFunctionType.Exp
    LN = mybir.ActivationFunctionType.Ln

    with TileContext(nc) as tc, ExitStack() as ctx:
        const = ctx.enter_context(tc.tile_pool(name="const", bufs=1))
        # weights as quarter tiles so the d-loop can start before the full
        # weight load lands; DMA emission order below is the arrival order.
        wq_sb = [const.tile([128, 4 * 512], bf16, name=f"wq{q}")
                 for q in range(8)]
        wk_sb = [const.tile([128, 8 * 128], bf16, name=f"wk{q}")
                 for q in range(4)]
        wv_sb = [const.tile([128, 8 * 128], bf16, name=f"wv{q}")
                 for q in range(4)]
        constsb = const.tile([HD, 385], bf16)
        masks = const.tile([HD, 4 * 512], bf16)
        maskneg = const.tile([HD, 4 * 512], bf16)
        constsf = const.tile([HD, 6], f32)
        cosT = const.tile([HD, S], bf16)
        sinT = const.tile([HD, S], bf16)
        wo_sb = const.tile([128, QH * D], bf16)
        rmat = constsb[:, 0:128]
        ident = constsb[:, 128:256]
        ones_col = constsb[:, 256:257]
        ones_row = constsb[0:1, 257:385]
        ksT = constsf[:, 0:5]
        ones_colr_t = const.tile([HD, 1], f32r)
        ones_colr = ones_colr_t[:]

        wq_c0 = const.tile([128, 512], bf16)
        x_c0 = const.tile([128, 512], bf16)
        # minimal start-gating set: first-chunk weights + x (0.5MB), then
        # the rest in order of first use
        nc.sync.dma_start(wq_c0[:], wqc0_d[:, :])
        nc.sync.dma_start(x_c0[:], xc0_d[:, :])
        nc.sync.dma_start(wq_sb[0][:], wq_d[:, 0:4 * 512])

        persist = ctx.enter_context(tc.tile_pool(name="persist", bufs=1))
        krot = persist.tile([HD, S], bf16)
        vnat = persist.tile([HD, NSK * HD], bf16)  # v tile i at cols i*128

        # PSUM: 8 banks as 4 singles (S0-S3, [128,512]) + 2 doubles
        # (D0/D1, [128,1024] = 2 banks each). Phases time-share by tag:
        #   A(j): accs/rope/vT entirely on S0-S2, so A(j+1) can run on the
        #         PE (as detour work) while B(j)'s exp backlog drains.
        #   B(j<3): scores head-paired on D0; av (both heads) on D1;
        #         dn on DVE; dn-reduce + recip-broadcast on S3.
        #   B(3): no A(4) detour exists, so keep the PE dense instead:
        #         scores depth-2 on D0/D1, av on S0/S1, dn on PE into S2.
        #   C(j): o accumulated in [128,1024] pairs on D0/D1.
        psum = ctx.enter_context(tc.tile_pool(name="psum", bufs=1, space="PSUM"))
        xpool = ctx.enter_context(tc.tile_pool(name="xpool", bufs=8))
        qrotp = ctx.enter_context(tc.tile_pool(name="qrotp", bufs=2))
        attnp = ctx.enter_context(tc.tile_pool(name="attnp", bufs=2))
        stage = ctx.enter_context(tc.tile_pool(name="stage", bufs=2))
        epool = ctx.enter_context(tc.tile_pool(name="epool", bufs=6))
        dnp = ctx.enter_context(tc.tile_pool(name="dnp", bufs=2))
        smallp = ctx.enter_context(tc.tile_pool(name="smallp", bufs=2))
        opool = ctx.enter_context(tc.tile_pool(name="opool", bufs=3))

        def pair2(tile, c0, c1):
            # [128, (2, c1-c0)] view over the two 512-wide halves of a
            # [128, 1024] tile
            return tile[:].rearrange("p (s w) -> p s w", s=2)[:, :, c0:c1]

        for j in range(J):
            sq = slice(j * 512, (j + 1) * 512)

            # ---------------- Phase A(j): projections + RoPE + V -----------
            xq = [xpool.tile([128, 4 * 512], bf16, tag="x", name=f"x{e}")
                  for e in range(XE)]
            if j == 0:
                # ladder: interleave x / wq eighths in consumption order so
                # the cold d-loop advances with DMA arrival; then the rest.
                nc.sync.dma_start(xq[0][:], xh_d[j, 0])
                nc.sync.dma_start(wq_sb[1][:], wq_d[:, 2048:4096])
                nc.sync.dma_start(xq[1][:], xh_d[j, 1])
                nc.sync.dma_start(constsb[:], constsb_d[:, :])
                nc.sync.dma_start(constsf[:], constsf_d[:, :])
                nc.sync.dma_start(wk_sb[0][:], wk_d[:, 0:8 * 128])
                nc.sync.dma_start(wv_sb[0][:], wv_d[:, 0:8 * 128])
                nc.sync.dma_start(ones_colr_t[:], onescr_d[:, :])
                for e in range(2, XE):
                    nc.sync.dma_start(wq_sb[e][:],
                                      wq_d[:, e * 2048:(e + 1) * 2048])
                    if e % 2 == 0:
                        qtr = e // 2
                        nc.sync.dma_start(
                            wk_sb[qtr][:],
                            wk_d[:, qtr * 1024:(qtr + 1) * 1024])
                        nc.sync.dma_start(
                            wv_sb[qtr][:],
                            wv_d[:, qtr * 1024:(qtr + 1) * 1024])
                    nc.sync.dma_start(xq[e][:], xh_d[j, e])
                nc.sync.dma_start(cosT[:], cosT_d[:, :])
                nc.sync.dma_start(sinT[:], sinT_d[:, :])
                nc.sync.dma_start(masks[:], masks_d[:, :])
                nc.sync.dma_start(maskneg[:], maskneg_d[:, :])
                nc.sync.dma_start(wo_sb[:], wo_d[:, :])
            else:
                for e in range(XE):
                    nc.gpsimd.dma_start(xq[e][:], xh_d[j, e])

            # Phase A(j): two d-passes (q0-q2, then q3/k/v) on S0-S2 only
            qrot_j = []
            v_sb = None
            for half in range(2):
                accs = [psum.tile([128, 512], f32, tag=f"S{a}",
                                  name=f"acc{half}_{a}") for a in range(3)]
                for c in range(DCH):
                    cq, c8 = c // 8, c % 8
                    ce, c4 = c // 4, c % 4
                    first = (j == 0 and c == 0)
                    xt = x_c0[:] if first \
                        else xq[ce][:, c4 * 512:(c4 + 1) * 512]
                    wqt = wq_c0 if first else wq_sb[ce]
                    wq_off = 0 if first else c4 * 512
                    st = (c == 0)
                    sp = (c == DCH - 1)
                    if half == 0:
                        for m in range(3):
                            nc.tensor.matmul(
                                accs[m][:],
                                wqt[:, wq_off + m * 128:
                                    wq_off + (m + 1) * 128],
                                xt, start=st, stop=sp)
                    else:
                        nc.tensor.matmul(
                            accs[0][:],
                            wqt[:, wq_off + 3 * 128:wq_off + 4 * 128],
                            xt, start=st, stop=sp)
                        nc.tensor.matmul(accs[1][:],
                                         wk_sb[cq][:, c8 * 128:(c8 + 1) * 128],
                                         xt, start=st, stop=sp)
                        nc.tensor.matmul(accs[2][:],
                                         wv_sb[cq][:, c8 * 128:(c8 + 1) * 128],
                                         xt, start=st, stop=sp)

                # RoPE (q0-q2 after half 0; q3,k after half 1; v copy only)
                srcs = [0, 1, 2] if half == 0 else [3, 4]
                for m in srcs:
                    a = m if half == 0 else m - 3
                    q_sb = stage.tile([128, 512], bf16, tag="qsb")
                    nc.scalar.copy(q_sb[:], accs[a][:])
                    rot_ps = psum.tile([128, 512], f32, tag=f"S{m % 3}",
                                       name=f"rot{m}")
                    nc.tensor.matmul(rot_ps[:], rmat, q_sb[:],
                                     start=True, stop=True)
                    t1 = stage.tile([128, 512], bf16, tag="t1")
                    nc.vector.tensor_mul(t1[:], q_sb[:], cosT[:, sq])
                    t2 = stage.tile([128, 512], bf16, tag="t2")
                    nc.vector.tensor_mul(t2[:], rot_ps[:], sinT[:, sq])
                    if m < QH:
                        dst = qrotp.tile([128, 512], bf16, tag=f"q{m}",
                                         name=f"qr{m}")
                        qrot_j.append(dst)
                        nc.vector.tensor_add(dst[:], t1[:], t2[:])
                    else:
                        nc.vector.tensor_add(krot[:, sq], t1[:], t2[:])
                if half == 1:
                    v_sb = stage.tile([128, 512], bf16, tag="vsb")
                    nc.scalar.copy(v_sb[:], accs[2][:])

            # V transpose into vnat (natural [sk, hd] layout)
            for b in range(4):
                i = 4 * j + b
                vt_ps = psum.tile([128, 512], bf16, tag=f"S{(2 + b) % 3}",
                                  name=f"vt{b}")
                nc.tensor.transpose(vt_ps[:, 0:128],
                                    v_sb[:, b * 128:(b + 1) * 128], ident)
                nc.vector.tensor_copy(vnat[:, i * 128:(i + 1) * 128],
                                      vt_ps[:, 0:128])

            # ---------------- Phase B(j): attention, head-paired ----------
            attn_j = {}
            ni = 4 * j + 4            # sk tiles 0..4j+3 are live
            last = J - 1

            def b_tile(i, h0, h1, s2, e2, av0, av1, dn_acc2):
                """score + exp(+scale) + mask + dn + av for sk tile i of the
                head pair (h0,h1); the two heads land in the two halves of
                s2/e2."""
                delta = i - 4 * j
                # causal trim: diagonal tile delta needs cols >= 128*delta
                c0 = 128 * delta if delta > 0 else 0
                cs0 = slice(c0, 512)
                cs1 = slice(512 + c0, 1024)
                ki = krot[:, i * 128:(i + 1) * 128]
                mm_mask = (j == J - 1 and delta >= 0)
                nc.tensor.matmul(s2[:, cs0], ki, qrot_j[h0][:, cs0],
                                 start=True, stop=not mm_mask)
                nc.tensor.matmul(s2[:, cs1], ki, qrot_j[h1][:, cs0],
                                 start=True, stop=not mm_mask)
                if mm_mask:
                    # additive -1e30 causal mask on the PE itself: skips two
                    # cross-engine hops in the only region with no detour
                    mneg = maskneg[:, delta * 512 + c0:(delta + 1) * 512]
                    nc.tensor.matmul(s2[:, cs0], ident, mneg,
                                     start=False, stop=True)
                    nc.tensor.matmul(s2[:, cs1], ident, mneg,
                                     start=False, stop=True)
                # enhance/suppress folded into exp's per-partition scale;
                # one 2D-AP activation covers both heads' halves
                if i < 5 and j >= 1:
                    if j == 1:
                        cb = BOUND - 512        # 99
                        nc.scalar.activation(pair2(e2, 0, cb),
                                             pair2(s2, 0, cb), EXP)
                        nc.scalar.activation(pair2(e2, cb, 512),
                                             pair2(s2, cb, 512), EXP,
                                             scale=ksT[:, i:i + 1])
                    else:
                        nc.scalar.activation(pair2(e2, c0, 512),
                                             pair2(s2, c0, 512), EXP,
                                             scale=ksT[:, i:i + 1])
                else:
                    nc.scalar.activation(pair2(e2, c0, 512),
                                         pair2(s2, c0, 512), EXP)
                if delta >= 0 and not mm_mask:
                    mki = masks[:, delta * 512 + c0:(delta + 1) * 512]
                    nc.vector.tensor_mul(e2[:, cs0], e2[:, cs0], mki)
                    nc.vector.tensor_mul(e2[:, cs1], e2[:, cs1], mki)
                st = (i == 0)
                sp = (i == ni - 1)
                # denominator accumulates off the PE; at j=3 the second
                # head-pair chain uses the otherwise-idle gpsimd engine
                dn_eng = nc.gpsimd if (j == J - 1 and h0 == 2) else nc.vector
                if i == 0:
                    dn_eng.tensor_copy(dn_acc2[:], e2[:])
                else:
                    dn_eng.tensor_add(pair2(dn_acc2, c0, 512),
                                      pair2(dn_acc2, c0, 512),
                                      pair2(e2, c0, 512))
                vi = vnat[:, i * 128:(i + 1) * 128]
                nc.tensor.matmul(av0[:, cs0], vi, e2[:, cs0],
                                 start=st, stop=sp)
                nc.tensor.matmul(av1[:, cs0 if av1 is not av0 else cs1], vi,
                                 e2[:, cs1], start=st, stop=sp)

            def b_tail(h0, h1, dn_acc2, av_of, rbtag):
                """dn partition-reduce, 1/x = exp(-ln(x)), K=1 broadcast,
                and the normalize-multiply for the pair (h0,h1)."""
                recs = []
                for hh in range(2):
                    dnr = psum.tile([1, 512], f32, tag=rbtag, name="dnr")
                    nc.tensor.matmul(dnr[:], ones_colr,
                                     dn_acc2[:, hh * 512:(hh + 1) * 512],
                                     start=True, stop=True)
                    lrec = smallp.tile([1, 512], f32, tag="lrec")
                    nc.scalar.activation(lrec[:], dnr[:], LN)
                    rec2 = smallp.tile([1, 512], bf16, tag="rec2")
                    nc.scalar.activation(rec2[:], lrec[:], EXP, scale=-1.0)
                    recs.append(rec2)
                for hh, h in enumerate((h0, h1)):
                    rb_ps = psum.tile([128, 512], f32, tag=rbtag, name="rb")
                    nc.tensor.matmul(rb_ps[:], ones_row,
                                     recs[hh][:], start=True, stop=True)
                    rb_sb = stage.tile([128, 512], f32, tag="rb")
                    nc.vector.tensor_copy(rb_sb[:], rb_ps[:])
                    at = attnp.tile([128, 512], bf16, tag=f"at{h}",
                                    name=f"at{h}")
                    nc.vector.tensor_mul(at[:], av_of(hh), rb_sb[:])
                    attn_j[h] = at

            if j < last:
                # head pairs sequentially; the PE detours into A(j+1)
                # whenever the exp backlog stalls this pair's pipeline
                for h0 in (0, 2):
                    av2 = psum.tile([128, 1024], f32, tag="D1", name="av2")
                    dn_acc2 = dnp.tile([128, 1024], f32r, tag="dn")
                    for i in range(ni):
                        s2 = psum.tile([128, 1024], f32, tag="D0", name="s2")
                        e2 = epool.tile([128, 1024], bf16, tag="e")
                        b_tile(i, h0, h0 + 1, s2, e2, av2, av2, dn_acc2)
                    b_tail(h0, h0 + 1, dn_acc2,
                           lambda hh, a=av2: a[:, hh * 512:(hh + 1) * 512],
                           "S3")
            else:
                # no A(4) exists to detour into: run BOTH head pairs as
                # independent interleaved pipelines (D0/D1 score chains,
                # av on all four single banks) to keep the PE dense
                av4 = [psum.tile([128, 512], f32, tag=f"S{k}", name=f"av{k}")
                       for k in range(4)]
                dn_accs = [dnp.tile([128, 1024], f32r, tag="dn",
                                    name=f"dna{p}") for p in range(2)]
                for i in range(ni):
                    for p, h0 in enumerate((0, 2)):
                        s2 = psum.tile([128, 1024], f32, tag=f"D{p}",
                                       name="s2")
                        e2 = epool.tile([128, 1024], bf16, tag="e")
                        b_tile(i, h0, h0 + 1, s2, e2,
                               av4[2 * p], av4[2 * p + 1], dn_accs[p])
                for p, h0 in enumerate((0, 2)):
                    b_tail(h0, h0 + 1, dn_accs[p],
                           lambda hh, p=p: av4[2 * p + hh][:], f"D{p}")

            # ---------------- Phase C(j): o_proj in n-pairs ---------------
            for t4 in range(4):
                t = 4 * j + t4
                ts_ = slice(t4 * 128, (t4 + 1) * 128)
                for np_ in range(4):
                    o2 = psum.tile([128, 1024], f32,
                                   tag=f"D{np_ % 2}", name="o2")
                    for h in range(QH):
                        for half in range(2):
                            n = 2 * np_ + half
                            nc.tensor.matmul(
                                o2[:, half * 512:(half + 1) * 512],
                                attn_j[h][:, ts_],
                                wo_sb[:, h * D + n * 512:h * D + (n + 1) * 512],
                                start=(h == 0), stop=(h == QH - 1))
                    o_sb = opool.tile([128, 1024], bf16, tag="osb")
                    nc.vector.tensor_copy(o_sb[:], o2[:])
                    nc.sync.dma_start(out[t, np_], o_sb[:])

    # Split multi-wait instructions (self-loading f32r matmuls allow only
    # one sync wait) onto standalone EventSemaphore instructions.
    import bass_rust
    bass_rust.generate_event_semaphores(nc)
    return nc


def _get_compiled():
    if "nc" not in _CACHE:
        _CACHE["nc"] = _build_bass()
        _CACHE["const"] = _host_constants()
    return _CACHE["nc"], _CACHE["const"]


def kernel(hidden_states, wq, wk, wv, wo, _trace=False):
    import ml_dtypes
    from concourse.bass_utils import run_bass_kernel_spmd

    bf16 = ml_dtypes.bfloat16
    nc, cst = _get_compiled()

    x = np.asarray(hidden_states, dtype=np.float32).reshape(S, D)
    xT = np.ascontiguousarray(x.T)                       # [D, S]
    # xh[j, q, p, 512*c8 + f] = xT[128*(8q+c8)+p, 512j+f]
    xh = np.ascontiguousarray(
        xT.reshape(XE, 4, 128, J, 512).transpose(3, 0, 2, 1, 4)
        .reshape(J, XE, 128, 4 * 512)).astype(bf16)
    wq = np.asarray(wq, dtype=np.float32)
    wk = np.asarray(wk, dtype=np.float32)
    wv = np.asarray(wv, dtype=np.float32)
    wo = np.asarray(wo, dtype=np.float32)
    scale = 1.0 / math.sqrt(HD)

    in_maps = []
    for d in range(NCORES):
        wq_d = wq[d * QH * HD:(d + 1) * QH * HD] * scale      # [512, D]
        wqT = np.ascontiguousarray(wq_d.T)                     # [D, 512]
        wq_sb = (wqT.reshape(DCH, 128, QH, 128).transpose(1, 0, 2, 3)
                 .reshape(128, DCH * 512)).astype(bf16)
        wkT = wk[d * HD:(d + 1) * HD].T                        # [D, 128]
        wk_sb = (wkT.reshape(DCH, 128, 128).transpose(1, 0, 2)
                 .reshape(128, DCH * 128)).astype(bf16)
        wvT = wv[d * HD:(d + 1) * HD].T
        wv_sb = (wvT.reshape(DCH, 128, 128).transpose(1, 0, 2)
                 .reshape(128, DCH * 128)).astype(bf16)
        woT = wo[:, d * QH * HD:(d + 1) * QH * HD].T           # [512, D]
        wo_sb = (woT.reshape(QH, 128, D).transpose(1, 0, 2)
                 .reshape(128, QH * D)).astype(bf16)
        in_maps.append({
            "xh": xh,
            "wq_c0": np.ascontiguousarray(wq_sb[:, 0:512]),
            "x_c0": np.ascontiguousarray(xh[0, 0][:, 0:512]),
            "wq": np.ascontiguousarray(wq_sb),
            "wk": np.ascontiguousarray(wk_sb),
            "wv": np.ascontiguousarray(wv_sb),
            "wo": np.ascontiguousarray(wo_sb),
            "cosT": cst["cosT"], "sinT": cst["sinT"],
            "masks": cst["masks"], "maskneg": cst["maskneg"],
            "constsb": cst["constsb"],
            "constsf": cst["constsf"], "ones_colr": cst["ones_colr"],
        })

    res = run_bass_kernel_spmd(nc, in_maps, core_ids=list(range(NCORES)),
                               trace=_trace)
    acc = res.results[0]["out"].astype(np.float32)
    for d in range(1, NCORES):
        acc += res.results[d]["out"].astype(np.float32)
    # out_t[t, n, p, f] -> out[128t+p, 512n+f]
    outp = acc.transpose(0, 2, 1, 3).reshape(S, D).astype(np.float32)
    outp = outp.reshape(1, S, D)
    if _trace:
        _CACHE["last_results"] = res
    return outp
